# revision 21
# baseline (speedup 1.0000x reference)
"""Trainium2 Bass kernel for nn_Att_cat_withid_norm (gnn_message_passing).

Math (B=2, N=512, D=64):
    value[b,i,j,:]  = ua[b,i,:] * ua[b,j,:]
    scores[b,i,j]   = leaky_relu( LN(ua[b,i])@w1 + LN(ua[b,j])@w2 + LN(iid[b])@w3 + b_att )
    alphas[b,i,j,:] = softmax_j(scores) broadcast over d
Scores are rank-1: scores[i,j] = lrelu(sq[i] + sk[j] + c).

Sharding: 8 cores = B(2) x 4 row-chunks of 128 query rows. Each core writes
[128, 512, 64] f32 for both outputs (2 x 16 MiB) -> HBM-write-bound
(roofline ~94 us/core at ~358 GB/s).

Engine split (raw bass, manual semaphores; the Tile framework's tail drain
does not compile on this walrus build):
  PE   - partition-broadcasts key data via ones-outer-product matmuls into
         PSUM (exact: 1.0 * k in fp32) + 2 small broadcast matmuls.
  DVE  - LN/softmax small ops, then value = rows(free-bcast AP) * kb(PSUM).
  ACT  - 3 sqrt + fused exp/rowsum, then alphas broadcast-normalize
         (step-0 inner AP, scale = 1/rowsum); issues alpha output DMAs.
  SP   - input loads + value output DMAs (HWDGE).
  POOL - tiny skey transpose bounce via DRAM (SWDGE).
"""

import numpy as np

B, N, D = 2, 512, 64
P = 128            # query rows per core
NCORES = 8
EPS = 1e-5
SLOPE = 0.01

KC = 64            # keys per output chunk
CH = KC * D        # 4096 free elems per output chunk
NT = N // KC       # 8 output chunks
KB = 1024          # PSUM broadcast tile free size (16 keys)
NU = CH // KB      # 4 kb tiles per chunk
NK = NT * NU       # 32 kb tiles total
KF = 8192          # keys-flat staging tile (covers 8 kb tiles)
NKF = N * D // KF  # 4 staging loads

_CACHE = {}
DEBUG = False


def _build(reps=1):
    key = ("nc", reps)
    if key in _CACHE:
        return _CACHE[key]

    from contextlib import ExitStack
    import concourse.bass as bass
    import concourse.mybir as mybir

    fp32 = mybir.dt.float32
    AX = mybir.AxisListType
    OP = mybir.AluOpType
    AF = mybir.ActivationFunctionType

    nc = bass.Bass("TRN2", target_bir_lowering=False, debug=False)

    rows_d = nc.dram_tensor("rows", [P, D], fp32, kind="ExternalInput")
    keys_d = nc.dram_tensor("keys", [N, D], fp32, kind="ExternalInput")
    aux_d = nc.dram_tensor("aux", [1, 448], fp32, kind="ExternalInput")
    ones_d = nc.dram_tensor("onesv", [1, P], fp32, kind="ExternalInput")
    outa_d = nc.dram_tensor("out_a", [P, N, D], fp32, kind="ExternalOutput")
    outv_d = nc.dram_tensor("out_v", [P, N, D], fp32, kind="ExternalOutput")
    skd_d = nc.dram_tensor("skd", [N], fp32)
    if DEBUG:
        dbg_d = nc.dram_tensor("dbg", [12, N], fp32, kind="ExternalOutput")

    keys_flat = keys_d.ap().flatten().unsqueeze(0)

    # --- DVE op counter milestones (op index == sem value after the op) ---
    D_BSRC = 16        # bsrc ready
    D_SKEY = 36        # skey ready
    D_NMAX = 39        # scores+nmax ready
    D_RINV = 42        # rinv ready
    D_VAL0 = 43        # value op for kb tile k is DVE op D_VAL0+k -> after: 43+k
    # ACT: 1=iln 2=irstd 3=rln 4=rrstd 5=kln 6=krstd 7=esb/sume 8=sln 9=rinv0
    # PE:  1=consts MM, 2=sk MM, then 2 MMs per kb tile: after tile k -> 4+2k
    TD = D_VAL0 + NK - 1   # DVE ops per rep (74)
    TP = 2 + 2 * NK        # PE ops per rep (66)
    TA = 9 + NT            # ACT ops per rep (17)

    def pe_after(gk):      # SPE count after global kb tile gk
        return (gk // NK) * TP + 4 + 2 * (gk % NK)

    def dve_after_val(gk):  # SDVE count after global value op gk
        return (gk // NK) * TD + D_VAL0 + (gk % NK)

    with ExitStack() as ctx:
        sb = lambda shape: ctx.enter_context(nc.sbuf_tensor(shape, fp32))
        ps = lambda shape: ctx.enter_context(nc.psum_tensor(shape, fp32))
        sem = lambda name: ctx.enter_context(nc.semaphore(name))

        rows = sb([P, D])
        keys3 = sb([P, 4 * D])
        aux = sb([1, 448])
        ones = sb([1, P])
        kf = [sb([1, KF]) for _ in range(2)]
        vt = [sb([P, CH]) for _ in range(3)]
        at = [sb([P, CH]) for _ in range(3)]

        gw = sb([1, 192]); bw = sb([1, 192]); cb = sb([1, 3])
        ism = sb([1, 1]); imean = sb([1, 1]); ixc = sb([1, D]); isq = sb([1, D])
        ivs = sb([1, 1]); ivar = sb([1, 1]); ivrec = sb([1, 1]); irstd = sb([1, 1])
        iscr = sb([1, D]); idot = sb([1, 1]); cbb = sb([1, 1]); base = sb([1, 1])
        bsrc = sb([1, 130]); bc = sb([P, 130])
        rsm = sb([P, 1]); rmean = sb([P, 1]); rxc = sb([P, D]); rsq = sb([P, D])
        rvs = sb([P, 1]); rvar = sb([P, 1]); rvrec = sb([P, 1]); rrstd = sb([P, 1])
        rscr = sb([P, D]); rdot = sb([P, 1]); srow = sb([P, 1])
        ksm = sb([P, 4]); kmean = sb([P, 4]); kxc = sb([P, 4 * D]); ksq = sb([P, 4 * D])
        kvs = sb([P, 4]); kvar = sb([P, 4]); kvrec = sb([P, 4]); krstd = sb([P, 4])
        kpr = sb([P, 4 * D]); kdot = sb([P, 4]); kmul = sb([P, 4]); skey = sb([P, 4])
        skf = sb([1, N])
        pre = sb([P, N]); scores = sb([P, N]); nmax = sb([P, 1])
        esb = sb([P, N]); sume = sb([P, 1]); rinv = sb([P, 1])

        kb = [ps([P, KB]) for _ in range(3)]
        cps = ps("cps", [P, 512])   # consts broadcast
        sps = ps("sps", [P, 512])   # sk broadcast

        SIN = sem("s_in"); SKF = sem("s_kf"); SKD = sem("s_skd")
        SVO = sem("s_vo"); SAO = sem("s_ao")
        SPE = sem("s_pe"); SDVE = sem("s_dve"); SACT = sem("s_act")

        g = aux.ap()[0:1, 0:64]
        lb = aux.ap()[0:1, 64:128]
        iid = aux.ap()[0:1, 128:192]
        w3r = aux.ap()[0:1, 192:384]
        batt = aux.ap()[0:1, 384:385]
        gw1b = bc.ap()[:, 0:64]
        gw2b = bc.ap()[:, 64:128]
        q1b = bc.ap()[:, 128:129]
        q2b = bc.ap()[:, 129:130]

        def v3(tile_ap):  # [P, n*D] -> [P, n, D]
            return tile_ap.rearrange("p (j d) -> p j d", d=D)

        k3view = keys3.ap().rearrange("p (c d) -> p c d", c=4)
        rows_b = rows.ap().unsqueeze(1).broadcast_to([P, KB // D, D])

        with nc.Block() as block:

            # ---------------- SP: input loads + value output DMAs ----------
            @block.sync
            def _(sp):
                sp.dma_start(rows.ap(), rows_d.ap()).then_inc(SIN, 16)
                sp.dma_start(
                    k3view, keys_d.ap().rearrange("(c p) d -> p c d", p=P)
                ).then_inc(SIN, 16)
                sp.dma_start(aux.ap(), aux_d.ap()).then_inc(SIN, 16)
                sp.dma_start(ones.ap(), ones_d.ap()).then_inc(SIN, 16)
                for c in range(2):
                    sp.dma_start(
                        kf[c].ap(), keys_flat[0:1, c * KF:(c + 1) * KF]
                    ).then_inc(SKF, 16)
                if DEBUG:
                    sp.wait_ge(SDVE, D_RINV)
                    sp.wait_ge(SACT, 9)
                    col = lambda r: dbg_d.ap()[r, :].rearrange("(p o) -> p o", o=1)[0:128, :]
                    sp.dma_start(dbg_d.ap()[0:1, :], esb.ap()[0:1, :]).then_inc(SIN, 16)
                    sp.dma_start(dbg_d.ap()[1:2, :], esb.ap()[1:2, :]).then_inc(SIN, 16)
                    sp.dma_start(col(2), sume.ap()).then_inc(SIN, 16)
                    sp.dma_start(col(3), rinv.ap()).then_inc(SIN, 16)
                    sp.dma_start(col(4), rinv0.ap()).then_inc(SIN, 16)
                    sp.dma_start(col(5), srow.ap()).then_inc(SIN, 16)
                    sp.dma_start(dbg_d.ap()[6:7, :], scores.ap()[0:1, :]).then_inc(SIN, 16)
                    sp.dma_start(dbg_d.ap()[7:8, :], pre.ap()[0:1, :]).then_inc(SIN, 16)
                    sp.dma_start(dbg_d.ap()[8:9, 0:4], kvar.ap()[0:1, :]).then_inc(SIN, 16)
                    sp.dma_start(dbg_d.ap()[9:10, 0:4], krstd.ap()[0:1, :]).then_inc(SIN, 16)
                    sp.dma_start(dbg_d.ap()[10:11, 0:4], kvs.ap()[0:1, :]).then_inc(SIN, 16)
                    sp.dma_start(dbg_d.ap()[11:12, 0:256], ksq.ap()[0:1, :]).then_inc(SIN, 16)
                for t in range(NT):
                    # value chunk t complete after DVE op D_VAL0 + 4t + 3
                    sp.wait_ge(SDVE, D_VAL0 + 4 * t + 3)
                    sp.dma_start(
                        outv_d.ap()[:, t * KC:(t + 1) * KC, :], v3(vt[t % 3].ap())
                    ).then_inc(SVO, 16)
                    if t in (0, 2):
                        c = t // 2 + 2
                        # kf buffer c%2 free once PE finished kb tile 8(c-2)+7
                        sp.wait_ge(SPE, 4 + 2 * (8 * (c - 2) + 7))
                        sp.dma_start(
                            kf[c % 2].ap(), keys_flat[0:1, c * KF:(c + 1) * KF]
                        ).then_inc(SKF, 16)

            # ---------------- PE: broadcast matmuls ------------------------
            @block.tensor
            def _(pe):
                for rep in range(reps):
                    OD = rep * TD
                    OKD = rep * 32
                    pe.wait_ge(SDVE, OD + D_BSRC)
                    pe.matmul(cps.ap()[:, 0:130], ones.ap(), bsrc.ap()).then_inc(SPE, 1)
                    pe.wait_ge(SKD, OKD + 32)
                    pe.matmul(sps.ap(), ones.ap(), skf.ap()).then_inc(SPE, 1)
                    for k in range(NK):
                        gk = rep * NK + k
                        c = k // 8
                        gc = rep * NKF + c
                        pe.wait_ge(SKF, 16 * (gc + 1))
                        if gk >= 3:
                            pe.wait_ge(SDVE, dve_after_val(gk - 3))
                        o = (k % 8) * KB
                        slot = kb[gk % 3].ap()
                        pe.matmul(slot[:, 0:512], ones.ap(), kf[c % 2].ap()[0:1, o:o + 512]).then_inc(SPE, 1)
                        pe.matmul(slot[:, 512:1024], ones.ap(), kf[c % 2].ap()[0:1, o + 512:o + 1024]).then_inc(SPE, 1)

            # ---------------- DVE: small chain + value ---------------------
            @block.vector
            def _(dv):
                dv.wait_ge(SIN, 64)
                g3 = lambda ap: ap.rearrange("p (k d) -> p k d", k=3)
                dv.tensor_tensor(g3(gw.ap()), g3(w3r), g.unsqueeze(1).broadcast_to([1, 3, 64]), op=OP.mult).then_inc(SDVE, 1)   # 1
                dv.tensor_tensor(g3(bw.ap()), g3(w3r), lb.unsqueeze(1).broadcast_to([1, 3, 64]), op=OP.mult).then_inc(SDVE, 1)  # 2
                dv.reduce_sum(cb.ap(), g3(bw.ap()), axis=AX.X).then_inc(SDVE, 1)    # 3
                dv.reduce_sum(ism.ap(), iid, axis=AX.X).then_inc(SDVE, 1)           # 4
                dv.tensor_scalar_mul(imean.ap(), ism.ap(), 1.0 / D).then_inc(SDVE, 1)  # 5
                dv.tensor_scalar_sub(ixc.ap(), iid, imean.ap()).then_inc(SDVE, 1)   # 6
                dv.tensor_tensor(isq.ap(), ixc.ap(), ixc.ap(), op=OP.mult).then_inc(SDVE, 1)  # 7
                dv.reduce_sum(ivs.ap(), isq.ap(), axis=AX.X).then_inc(SDVE, 1)      # 8
                dv.tensor_scalar(ivar.ap(), ivs.ap(), 1.0 / D, EPS, op0=OP.mult, op1=OP.add).then_inc(SDVE, 1)  # 9
                dv.tensor_tensor(iscr.ap(), ixc.ap(), gw.ap()[0:1, 128:192], op=OP.mult).then_inc(SDVE, 1)  # 10
                dv.reduce_sum(idot.ap(), iscr.ap(), axis=AX.X).then_inc(SDVE, 1)    # 11
                dv.tensor_tensor(cbb.ap(), cb.ap()[0:1, 2:3], batt, op=OP.add).then_inc(SDVE, 1)  # 12
                dv.wait_ge(SACT, 2)
                dv.tensor_scalar(base.ap(), idot.ap(), irstd.ap(), cbb.ap(), op0=OP.mult, op1=OP.add).then_inc(SDVE, 1)  # 13
                dv.tensor_copy(bsrc.ap()[0:1, 0:128], gw.ap()[0:1, 0:128]).then_inc(SDVE, 1)  # 14
                dv.tensor_tensor(bsrc.ap()[0:1, 128:129], base.ap(), cb.ap()[0:1, 0:1], op=OP.add).then_inc(SDVE, 1)  # 15
                dv.tensor_copy(bsrc.ap()[0:1, 129:130], cb.ap()[0:1, 1:2]).then_inc(SDVE, 1)  # 16 == D_BSRC
                dv.wait_ge(SPE, 1)
                dv.tensor_copy(bc.ap(), cps.ap()[:, 0:130]).then_inc(SDVE, 1)       # 17
                # rows LN
                dv.reduce_sum(rsm.ap(), rows.ap(), axis=AX.X).then_inc(SDVE, 1)     # 18
                dv.tensor_scalar_mul(rmean.ap(), rsm.ap(), 1.0 / D).then_inc(SDVE, 1)  # 19
                dv.tensor_scalar_sub(rxc.ap(), rows.ap(), rmean.ap()).then_inc(SDVE, 1)  # 20
                dv.tensor_tensor(rsq.ap(), rxc.ap(), rxc.ap(), op=OP.mult).then_inc(SDVE, 1)  # 21
                dv.reduce_sum(rvs.ap(), rsq.ap(), axis=AX.X).then_inc(SDVE, 1)      # 22
                dv.tensor_scalar(rvar.ap(), rvs.ap(), 1.0 / D, EPS, op0=OP.mult, op1=OP.add).then_inc(SDVE, 1)  # 23
                dv.tensor_tensor(rscr.ap(), rxc.ap(), gw1b, op=OP.mult).then_inc(SDVE, 1)  # 24
                dv.reduce_sum(rdot.ap(), rscr.ap(), axis=AX.X).then_inc(SDVE, 1)    # 25
                dv.wait_ge(SACT, 4)
                dv.tensor_scalar(srow.ap(), rdot.ap(), rrstd.ap(), q1b, op0=OP.mult, op1=OP.add).then_inc(SDVE, 1)  # 26
                # keys LN (segmented over 4 chunks)
                c4 = lambda ap: ap.rearrange("p (c d) -> p c d", c=4)
                dv.reduce_sum(ksm.ap(), k3view, axis=AX.X).then_inc(SDVE, 1)        # 27
                dv.tensor_scalar_mul(kmean.ap(), ksm.ap(), 1.0 / D).then_inc(SDVE, 1)  # 28
                dv.tensor_tensor(c4(kxc.ap()), k3view, kmean.ap().unsqueeze(2).broadcast_to([P, 4, D]), op=OP.subtract).then_inc(SDVE, 1)  # 29
                dv.tensor_tensor(c4(ksq.ap()), c4(kxc.ap()), c4(kxc.ap()), op=OP.mult).then_inc(SDVE, 1)  # 30
                dv.reduce_sum(kvs.ap(), c4(ksq.ap()), axis=AX.X).then_inc(SDVE, 1)  # 31
                dv.tensor_scalar(kvar.ap(), kvs.ap(), 1.0 / D, EPS, op0=OP.mult, op1=OP.add).then_inc(SDVE, 1)  # 32
                dv.tensor_tensor(c4(kpr.ap()), c4(kxc.ap()), gw2b.unsqueeze(1).broadcast_to([P, 4, D]), op=OP.mult).then_inc(SDVE, 1)  # 33
                dv.reduce_sum(kdot.ap(), c4(kpr.ap()), axis=AX.X).then_inc(SDVE, 1)  # 34
                dv.wait_ge(SACT, 6)
                dv.tensor_tensor(kmul.ap(), kdot.ap(), krstd.ap(), op=OP.mult).then_inc(SDVE, 1)  # 35
                dv.tensor_scalar_add(skey.ap(), kmul.ap(), q2b).then_inc(SDVE, 1)   # 36 == D_SKEY
                # scores + softmax stats
                dv.wait_ge(SPE, 2)
                dv.tensor_scalar_add(pre.ap(), sps.ap(), srow.ap()).then_inc(SDVE, 1)  # 37
                dv.scalar_tensor_tensor(scores.ap(), pre.ap(), SLOPE, pre.ap(), op0=OP.mult, op1=OP.max).then_inc(SDVE, 1)  # 38
                dv.reduce_max(nmax.ap(), scores.ap(), axis=AX.X, negate=True).then_inc(SDVE, 1)  # 39 == D_NMAX
                dv.wait_ge(SACT, 9)
                dv.tensor_tensor(nr1.ap(), sume.ap(), rinv0.ap(), op=OP.mult).then_inc(SDVE, 1)  # 40
                dv.tensor_scalar(nr2.ap(), nr1.ap(), -1.0, 2.0, op0=OP.mult, op1=OP.add).then_inc(SDVE, 1)  # 41
                dv.tensor_tensor(rinv.ap(), nr2.ap(), rinv0.ap(), op=OP.mult).then_inc(SDVE, 1)  # 42 == D_RINV
                # value ops
                for k in range(NK):
                    t, u = divmod(k, NU)
                    dv.wait_ge(SPE, 4 + 2 * k)
                    if u == 0 and t >= 3:
                        dv.wait_ge(SVO, 16 * (t - 2))
                    dv.tensor_tensor(
                        v3(vt[t % 3].ap()[:, u * KB:(u + 1) * KB]),
                        rows_b,
                        v3(kb[k % 3].ap()),
                        op=OP.mult,
                    ).then_inc(SDVE, 1)   # 44 + k

            # ---------------- ACT: sqrt/exp + alphas + alpha DMAs ----------
            @block.scalar
            def _(ac):
                ac.wait_ge(SDVE, 9)
                ac.activation(iln.ap(), ivar.ap(), AF.Ln).then_inc(SACT, 1)          # 1
                ac.activation(irstd.ap(), iln.ap(), AF.Exp, scale=-0.5).then_inc(SACT, 1)  # 2
                ac.wait_ge(SDVE, 23)
                ac.activation(rln.ap(), rvar.ap(), AF.Ln).then_inc(SACT, 1)          # 3
                ac.activation(rrstd.ap(), rln.ap(), AF.Exp, scale=-0.5).then_inc(SACT, 1)  # 4
                ac.wait_ge(SDVE, 32)
                ac.activation(kln.ap(), kvar.ap(), AF.Ln).then_inc(SACT, 1)          # 5
                ac.activation(krstd.ap(), kln.ap(), AF.Exp, scale=-0.5).then_inc(SACT, 1)  # 6
                ac.wait_ge(SDVE, D_NMAX)
                ac.activation(esb.ap(), scores.ap(), AF.Exp, bias=nmax.ap(), accum_out=sume.ap()).then_inc(SACT, 1)  # 7
                ac.activation(sln.ap(), sume.ap(), AF.Ln).then_inc(SACT, 1)          # 8
                ac.activation(rinv0.ap(), sln.ap(), AF.Exp, scale=-1.0).then_inc(SACT, 1)  # 9
                ac.wait_ge(SDVE, D_RINV)
                for t in range(NT):
                    if t >= 3:
                        ac.wait_ge(SAO, 16 * (t - 2))
                    ac.activation(
                        v3(at[t % 3].ap()),
                        esb.ap()[:, t * KC:(t + 1) * KC].unsqueeze(2).broadcast_to([P, KC, D]),
                        AF.Copy,
                        scale=rinv.ap(),
                    ).then_inc(SACT, 1)   # 5 + t
                    ac.dma_start(
                        outa_d.ap()[:, t * KC:(t + 1) * KC, :], v3(at[t % 3].ap())
                    ).then_inc(SAO, 16)

            # ---------------- POOL: skey transpose bounce ------------------
            @block.gpsimd
            def _(gp):
                gp.wait_ge(SDVE, D_SKEY)
                with nc.allow_non_contiguous_dma(reason="512x4B skey transpose"):
                    gp.dma_start(skd_d.ap().rearrange("(c p) -> p c", p=P), skey.ap()).then_inc(SKD, 16)
                gp.wait_ge(SKD, 16)
                gp.dma_start(skf.ap(), skd_d.ap().unsqueeze(0)).then_inc(SKD, 16)

    _CACHE[key] = nc
    return nc


def kernel(ua, iid, ln_g, ln_b, w_att, b_att, _trace=False, _trace_kwargs=None):
    from concourse.bass_utils import run_bass_kernel_spmd

    ua = np.ascontiguousarray(np.asarray(ua, dtype=np.float32))
    iid = np.asarray(iid, dtype=np.float32)
    ln_g = np.asarray(ln_g, dtype=np.float32)
    ln_b = np.asarray(ln_b, dtype=np.float32)
    w_att = np.asarray(w_att, dtype=np.float32)
    b_att = np.asarray(b_att, dtype=np.float32)

    nc = _build(1)

    onesv = np.ones((1, P), dtype=np.float32)
    in_maps = []
    for c in range(NCORES):
        b, rc = divmod(c, 4)
        aux = np.zeros((1, 448), dtype=np.float32)
        aux[0, 0:64] = ln_g
        aux[0, 64:128] = ln_b
        aux[0, 128:192] = iid[b, 0, 0]
        aux[0, 192:384] = w_att[:, 0]
        aux[0, 384] = b_att[0]
        in_maps.append(
            {
                "rows": np.ascontiguousarray(ua[b, rc * P:(rc + 1) * P]),
                "keys": ua[b],
                "aux": aux,
                "onesv": onesv,
            }
        )

    kw = {}
    if _trace:
        kw["trace"] = True
        kw.update(_trace_kwargs or {})
    r = run_bass_kernel_spmd(nc, in_maps, core_ids=list(range(NCORES)), **kw)
    _CACHE["last_result"] = r

    alphas = np.empty((B, N, N, D), dtype=np.float32)
    value = np.empty((B, N, N, D), dtype=np.float32)
    for c in range(NCORES):
        b, rc = divmod(c, 4)
        alphas[b, rc * P:(rc + 1) * P] = r.results[c]["out_a"]
        value[b, rc * P:(rc + 1) * P] = r.results[c]["out_v"]
    return alphas, value
